# revision 12
# baseline (speedup 1.0000x reference)
"""Trainium2 Bass kernel for nn_Network_38491496907327.

Computes, for X [65536, 512] f32 (with C1 = I, C2 = 1, C3 = 0 -- verified at
call time, exact-numpy fallback otherwise):

    quad = sum(X * X, axis=-1)                       # row-wise quadratic form
    y    = quad[:, None] + X
    out  = (y - mean_0(y)) / sqrt(var_0(y) + 1e-5)   # BatchNorm1d over axis 0

Distribution: data-parallel over rows, 8192 rows/core on 8 NeuronCores.
Batch statistics are reduced to five sufficient statistics per shard
(colsum X, colsum q'X, colsum X^2; sum q', sum q'^2 with q' = quad - 512)
and AllReduce-summed across cores; each core then normalizes its shard.

Per-core pipeline (64 row-tiles of [128, 512]):
  pass A: DMA X in (2MB super-chunks); ScalarE: round-copy X -> f32r
          (kept resident in SBUF) and Square(X) -> X^2 (f32r) with an exact
          fp32 row-sum accumulator (quad); TensorE (f32r, 1 cyc/row):
          PSUM-accumulated colsum matmuls ones@X, q'@X, ones@X^2.
  q-moment reduction via PE transpose + DVE row-reduce (exact fp32),
  AllReduce of a [2,1540] stats buffer, tiny vector math for mean/invstd,
  broadcasts materialized by K=1/K=2 PE outer products (hi/lo split keeps
  the invstd row at fp32 precision).
  pass B: PE: psum = I@X + ones128 (x) c   (c = -colmean(X));
          DVE: out = (psum + qc) * invstd  -- one fused scalar_tensor_tensor
          per tile (qc = per-row centered quad, an exact per-partition
          scalar); DMA out (2MB super-chunks).
"""

import sys

if "/opt/trn_rl_repo" not in sys.path:
    sys.path.insert(0, "/opt/trn_rl_repo")

import numpy as np

N = 65536
K = 512
NCORES = 8
ROWS = N // NCORES          # 8192 rows per core
P = 128                     # partitions
TILES = ROWS // P           # 64 row-tiles per core
SUP = 8                     # tiles per DMA super-chunk (2 MB)
NSUP = TILES // SUP         # 8 super-chunks
BN_EPS = 1e-5
QSHIFT = 512.0   # a-priori center of quad = ||x_row||^2 for x ~ N(0,1), K=512

_CACHE = {}


def _build(reps=1, serialize=True):
    from concourse import bacc, tile, mybir

    F32 = mybir.dt.float32
    F32R = mybir.dt.float32r
    ALU = mybir.AluOpType
    ACTF = mybir.ActivationFunctionType

    nc = bacc.Bacc("TRN2", target_bir_lowering=False, debug=False,
                   num_devices=NCORES)
    x_in = nc.dram_tensor("x", [ROWS, K], F32, kind="ExternalInput").ap()
    y_out = nc.dram_tensor("out", [ROWS, K], F32, kind="ExternalOutput").ap()
    ident_dram = nc.inline_tensor(np.eye(P, dtype=np.float32), name="ident")

    STATS_W = 1540  # 512 sx | 512 sqx | 512 sxx | 4 pad ; row1: sq, sqq

    with tile.TileContext(nc) as tc:
        with tc.tile_pool(name="sbuf", bufs=1) as pool, \
             tc.tile_pool(name="big", bufs=2) as bigpool, \
             tc.tile_pool(name="x2p", bufs=2) as x2pool, \
             tc.tile_pool(name="pps", bufs=1, space="PSUM") as pstat_pool, \
             tc.tile_pool(name="ppo", bufs=3, space="PSUM") as pout_pool, \
             tc.tile_pool(name="dram", bufs=1, space="DRAM") as dram:
            # ---- constants ----
            ident_f = pool.tile([P, P], F32)
            nc.sync.dma_start(out=ident_f[:], in_=ident_dram.ap())
            ident_r = pool.tile([P, P], F32R)
            nc.scalar.copy(ident_r[:], ident_f[:])

            onescol = pool.tile([P, 1], F32)
            nc.vector.memset(onescol[:], 1.0)
            onescol_r = pool.tile([P, 1], F32R)
            nc.vector.tensor_copy(onescol_r[:], onescol[:])
            onesrow = pool.tile([1, P], F32)
            nc.vector.memset(onesrow[:], 1.0)
            onesrow_r = pool.tile([1, P], F32R)
            nc.vector.tensor_copy(onesrow_r[:], onesrow[:])

            def body():
                # ---- per-iteration state (bufs=1 pools: stable addresses) --
                xr_all = pool.tile([P, TILES * K], F32R, tag="xr_all")
                q_all = pool.tile([P, TILES], F32, tag="q_all")
                qr_all = pool.tile([P, TILES], F32R, tag="qr_all")
                ps_sx = pstat_pool.tile([1, K], F32, tag="ps_sx")
                ps_sqx = pstat_pool.tile([1, K], F32, tag="ps_sqx")
                ps_sxx = pstat_pool.tile([1, K], F32, tag="ps_sxx")

                # ================= pass A =================
                for s in range(NSUP):
                    xsup = bigpool.tile([P, SUP * K], F32, tag="big")
                    dram_ap = x_in[s * SUP * P:(s + 1) * SUP * P, :] \
                        .rearrange("(p j) k -> p (j k)", p=P)
                    nc.sync.dma_start(out=xsup[:], in_=dram_ap)
                    for j in range(SUP):
                        t = s * SUP + j
                        xt = xsup[:, j * K:(j + 1) * K]
                        nc.vector.tensor_copy(xr_all[:, t * K:(t + 1) * K], xt)
                        x2 = x2pool.tile([P, K], F32R, tag="x2")
                        nc.scalar.activation(x2[:], xt, ACTF.Square,
                                             accum_out=q_all[:, t:t + 1])
                        nc.vector.tensor_scalar_add(qr_all[:, t:t + 1],
                                                    q_all[:, t:t + 1], -QSHIFT)
                        xr_t = xr_all[:, t * K:(t + 1) * K]
                        first = (t == 0)
                        last = (t == TILES - 1)
                        nc.tensor.matmul(ps_sx[:], onescol_r[:], xr_t,
                                         start=first, stop=last)
                        nc.tensor.matmul(ps_sqx[:], qr_all[:, t:t + 1], xr_t,
                                         start=first, stop=last)
                        nc.tensor.matmul(ps_sxx[:], onescol_r[:], x2[:],
                                         start=first, stop=last)

                # q' = quad - QSHIFT (exact; kills fp32 cancellation in
                # Var(q) since quad ~ QSHIFT)
                nc.vector.tensor_scalar_add(q_all[:], q_all[:], -QSHIFT)

                # ---- local q' moments, exact fp32:
                # free-axis reduce -> [128,2], PE-transpose -> [2,128],
                # free-axis reduce -> [2,1] ----
                qsq = pool.tile([P, 2], F32, tag="qsq")
                qscr = pool.tile([P, TILES], F32, tag="qscr")
                nc.vector.tensor_reduce(qsq[:, 0:1], q_all[:],
                                        mybir.AxisListType.X, ALU.add)
                nc.vector.scalar_tensor_tensor(
                    out=qscr[:], in0=q_all[:], scalar=1.0, in1=q_all[:],
                    op0=ALU.mult, op1=ALU.mult, accum_out=qsq[:, 1:2])
                qpad = pool.tile([P, P], F32, tag="qpad")
                nc.vector.memset(qpad[:], 0.0)
                nc.vector.tensor_copy(qpad[:, 0:2], qsq[:])
                pqt = pout_pool.tile([P, P], F32, tag="po")
                nc.tensor.matmul(pqt[:], qpad[:], ident_f[:],
                                 is_transpose=True)
                qtr = pool.tile([2, P], F32, tag="qtr")
                nc.scalar.copy(qtr[:], pqt[0:2, :])
                qsum = pool.tile([2, 1], F32, tag="qsum")
                nc.vector.tensor_reduce(qsum[:], qtr[:],
                                        mybir.AxisListType.X, ALU.add)

                # ---- stage stats -> AllReduce -> global stats ----
                staging = pool.tile([1, STATS_W], F32, tag="staging")
                nc.vector.memset(staging[:], 0.0)
                nc.scalar.copy(staging[:, 0:K], ps_sx[:])
                nc.scalar.copy(staging[:, K:2 * K], ps_sqx[:])
                nc.scalar.copy(staging[:, 2 * K:3 * K], ps_sxx[:])
                # move [2,1] q-sums (partitions 0,1) onto partition-0 columns
                # via a DRAM hop (engines cannot read partition 1 directly)
                qtmp = dram.tile([2, 1], F32, tag="qtmp")
                nc.sync.dma_start(out=qtmp[:], in_=qsum[:])
                nc.sync.dma_start(out=staging[:, 3 * K:3 * K + 2],
                                  in_=qtmp.opt().rearrange("a b -> b a"))
                bounce_in = dram.tile([1, STATS_W], F32, tag="b_in")
                bounce_out = dram.tile([1, STATS_W], F32, tag="b_out")
                nc.sync.dma_start(out=bounce_in[:], in_=staging[:])
                nc.gpsimd.collective_compute(
                    "AllReduce", ALU.add,
                    replica_groups=[list(range(NCORES))],
                    ins=[bounce_in.opt()], outs=[bounce_out.opt()])
                gst = staging
                nc.sync.dma_start(out=gst[:], in_=bounce_out[:])

                # ---- derived vectors (partition 0) ----
                invN = 1.0 / float(N)
                Sx = gst[:, 0:K]
                Sqx = gst[:, K:2 * K]
                Sxx = gst[:, 2 * K:3 * K]
                Sq = gst[:, 3 * K:3 * K + 1]
                Sqq = gst[:, 3 * K + 1:3 * K + 2]

                # var = Var(q') + 2*Cov(q',X) + Var(X), centered pieces
                qbar = pool.tile([1, 1], F32, tag="qbar")
                nc.vector.tensor_scalar_mul(qbar[:], Sq, invN)
                m2qbar = pool.tile([1, 1], F32, tag="m2qbar")
                nc.vector.tensor_scalar_mul(m2qbar[:], qbar[:], -2.0)
                s0 = pool.tile([1, 1], F32, tag="s0")   # Var(q')
                nc.vector.tensor_tensor(out=s0[:], in0=qbar[:], in1=qbar[:],
                                        op=ALU.mult)
                sqqn = pool.tile([1, 1], F32, tag="sqqn")
                nc.vector.tensor_scalar_mul(sqqn[:], Sqq, invN)
                nc.vector.tensor_sub(s0[:], sqqn[:], s0[:])
                xbar = pool.tile([1, K], F32, tag="xbar")
                nc.vector.tensor_scalar_mul(xbar[:], Sx, invN)
                t1 = pool.tile([1, K], F32, tag="t1")
                nc.vector.tensor_scalar_mul(t1[:], Sqx, 2.0 * invN)
                t2 = pool.tile([1, K], F32, tag="t2")
                nc.vector.scalar_tensor_tensor(
                    out=t2[:], in0=xbar[:], scalar=m2qbar[:], in1=t1[:],
                    op0=ALU.mult, op1=ALU.add)
                t3 = pool.tile([1, K], F32, tag="t3")
                nc.vector.tensor_scalar_mul(t3[:], Sxx, invN)
                x2b = pool.tile([1, K], F32, tag="x2b")
                nc.vector.tensor_tensor(out=x2b[:], in0=xbar[:], in1=xbar[:],
                                        op=ALU.mult)
                varv = pool.tile([1, K], F32, tag="varv")
                nc.vector.tensor_add(varv[:], t2[:], t3[:])
                nc.vector.tensor_sub(varv[:], varv[:], x2b[:])
                nc.vector.tensor_scalar(out=varv[:], in0=varv[:],
                                        scalar1=s0[:], scalar2=None,
                                        op0=ALU.add)
                epst = pool.tile([1, 1], F32, tag="epst")
                nc.vector.memset(epst[:], BN_EPS)
                sd = pool.tile([1, K], F32, tag="sd")
                nc.scalar.activation(sd[:], varv[:], ACTF.Sqrt, bias=epst[:])
                inv = pool.tile([1, K], F32, tag="inv")
                nc.vector.reciprocal(inv[:], sd[:])

                crow_r = pool.tile([1, K], F32R, tag="crow_r")  # c = -xbar
                nc.vector.tensor_scalar_mul(crow_r[:], Sx, -invN)

                # qc = q' - qbar': replicate qbar along the free axis on
                # partition 0, then DRAM-hop transpose to a [128,1] column
                qrow = pool.tile([1, P], F32, tag="qrow")
                nc.vector.tensor_scalar(out=qrow[:], in0=onesrow[:],
                                        scalar1=qbar[:], scalar2=None,
                                        op0=ALU.mult)
                qtmp2 = dram.tile([1, P], F32, tag="qtmp2")
                nc.sync.dma_start(out=qtmp2[:], in_=qrow[:])
                qbarb = pool.tile([P, 1], F32, tag="qbarb")
                nc.sync.dma_start(out=qbarb[:],
                                  in_=qtmp2.opt().rearrange("a b -> b a"))
                qc_all = pool.tile([P, TILES], F32, tag="qc_all")
                nc.vector.tensor_scalar_sub(qc_all[:], q_all[:], qbarb[:])

                # invstd broadcast to [128, K] via two K=1 outer products
                # (hi/lo split keeps fp32 precision through f32r operands)
                inv_hi = pool.tile([1, K], F32R, tag="inv_hi")
                nc.vector.tensor_copy(inv_hi[:], inv[:])
                inv_lo = pool.tile([1, K], F32, tag="inv_lo")
                nc.vector.tensor_sub(inv_lo[:], inv[:],
                                     inv_hi[:].bitcast(F32))
                inv_lo_r = pool.tile([1, K], F32R, tag="inv_lo_r")
                nc.vector.tensor_copy(inv_lo_r[:], inv_lo[:])
                pab = pout_pool.tile([P, K], F32, tag="po")
                nc.tensor.matmul(pab[:], onesrow_r[:], inv_hi[:],
                                 start=True, stop=False)
                nc.tensor.matmul(pab[:], onesrow_r[:], inv_lo_r[:],
                                 start=False, stop=True)
                abct = pool.tile([P, K], F32, tag="abct")
                nc.scalar.copy(abct[:], pab[:])

                # ================= pass B =================
                for s in range(NSUP):
                    osup = bigpool.tile([P, SUP * K], F32, tag="big")
                    for j in range(SUP):
                        t = s * SUP + j
                        xr_t = xr_all[:, t * K:(t + 1) * K]
                        pout = pout_pool.tile([P, K], F32, tag="po")
                        nc.tensor.matmul(pout[:], ident_r[:], xr_t,
                                         start=True, stop=False)
                        nc.tensor.matmul(pout[:], onesrow_r[:], crow_r[:],
                                         start=False, stop=True)
                        nc.vector.scalar_tensor_tensor(
                            out=osup[:, j * K:(j + 1) * K], in0=pout[:],
                            scalar=qc_all[:, t:t + 1], in1=abct[:],
                            op0=ALU.add, op1=ALU.mult)
                    dram_ap = y_out[s * SUP * P:(s + 1) * SUP * P, :] \
                        .rearrange("(p j) k -> p (j k)", p=P)
                    nc.sync.dma_start(out=dram_ap, in_=osup[:])

            for r in range(reps):
                if serialize and r > 0:
                    tc.strict_bb_all_engine_barrier()
                body()

    nc.compile()
    return nc


def _get_nc():
    if "nc" not in _CACHE:
        _CACHE["nc"] = _build()
    return _CACHE["nc"]


def _fallback(X, C1, C2, C3):
    X64 = X.astype(np.float64)
    quad = np.einsum("nk,kj,nj->n", X64, C1.astype(np.float64), X64)
    y = quad[:, None] + C2.astype(np.float64) * X64 + C3.astype(np.float64)
    mean = y.mean(axis=0)
    var = ((y - mean) ** 2).mean(axis=0)
    return ((y - mean) / np.sqrt(var + BN_EPS)).astype(np.float32)


def kernel(X, C1, C2, C3):
    X = np.ascontiguousarray(np.asarray(X, dtype=np.float32))
    C1 = np.asarray(C1, dtype=np.float32)
    C2 = np.asarray(C2, dtype=np.float32)
    C3 = np.asarray(C3, dtype=np.float32)
    fast = (
        X.shape == (N, K)
        and C1.shape == (K, K)
        and np.array_equal(C1, np.eye(K, dtype=np.float32))
        and C2.shape == (K,) and np.all(C2 == 1.0)
        and np.all(C3 == 0.0)
    )
    if not fast:
        return _fallback(X, C1, C2, C3)

    from concourse.bass_utils import run_bass_kernel_spmd

    nc = _get_nc()
    in_maps = [{"x": X[i * ROWS:(i + 1) * ROWS]} for i in range(NCORES)]
    res = run_bass_kernel_spmd(nc, in_maps, core_ids=list(range(NCORES)))
    return np.concatenate([res.results[i]["out"] for i in range(NCORES)], axis=0)
